# revision 23
# baseline (speedup 1.0000x reference)
"""Trainium2 Bass kernel for nn_FSE_Module_79147657331158.

Pipeline (per batch image, one per NeuronCore, 8-way data parallel):
  h1 = mish(BN1(conv3x3(x, w1)))          64 -> 128 ch
  h2 = mish(BN2(conv3x3(h1, w2))) + x     128 -> 64 ch
  cA, (cH,cV,cD) = haar_dwt2(h2)
  x_low  = cA
  x_high = mish(BNh(conv1x1(concat(cH,cV,cD), wh)))

Implementation notes:
  - convs are 9-tap (3x3) / 4-tap (2x2-stride-2, DWT-fused 1x1) matmul
    accumulations in PSUM, float32r (TF32-like, full PE rate at N>=256).
  - BN scale is folded into the weights host-side; BN bias is applied by
    the ACT engine during PSUM evacuation (Identity + per-partition bias).
  - mish(z) = z * (g-1)/(g+1) with g = (1+exp(z))^2: exp and square on
    the scalar engine (one table set), reciprocal via the fast custom DVE
    op, final muls on the vector engine; the +1 offset on GPSIMD.
  - The DWT + 1x1 conv are fused: x_high = mish(BNh(conv2x2s2(h2, W')))
    where W' combines wh with the Haar signs; x_low is computed with
    vector adds directly from h2.
  - h2 is stored column-deinterleaved ([even cols | odd cols] per row)
    and row-packed across partition halves so the 2x2-stride-2 conv taps
    and the DWT adds read contiguous spans at full 128-partition width.
"""
import os
import sys
from contextlib import ExitStack

sys.path.insert(0, "/opt/trn_rl_repo")

import numpy as np

_CACHE = {}


def _fold_params(w1, b1, g1, be1, m1, v1, w2, b2, g2, be2, m2, v2,
                 wh, bh, gh, beh, mh, vh):
    eps = 1e-5
    f64 = np.float64
    s1 = (g1.astype(f64) / np.sqrt(v1.astype(f64) + eps))
    bv1 = ((b1.astype(f64) - m1) * s1 + be1)
    w1t = (w1.astype(f64) * s1[:, None, None, None]).transpose(2, 3, 1, 0)
    w1t = np.ascontiguousarray(w1t.reshape(9, 64, 128), dtype=np.float32)

    s2 = (g2.astype(f64) / np.sqrt(v2.astype(f64) + eps))
    bv2 = ((b2.astype(f64) - m2) * s2 + be2)
    w2t = (w2.astype(f64) * s2[:, None, None, None]).transpose(2, 3, 1, 0)
    w2t = np.ascontiguousarray(w2t.reshape(9, 128, 64), dtype=np.float32)

    sh = (gh.astype(f64) / np.sqrt(vh.astype(f64) + eps))
    bvh = ((bh.astype(f64) - mh) * sh + beh)
    whm = wh[:, :, 0, 0].astype(f64)  # [64, 192]
    wH, wV, wD = whm[:, :64], whm[:, 64:128], whm[:, 128:]
    wpt = np.zeros((4, 128, 64), dtype=np.float32)
    for a in (0, 1):
        for b in (0, 1):
            sH = 1.0 if a == 0 else -1.0
            sV = 1.0 if b == 0 else -1.0
            sD = 1.0 if a == b else -1.0
            wp = 0.5 * (wH * sH + wV * sV + wD * sD) * sh[:, None]  # [o, c]
            wpt[2 * a + b, :64, :] = wp.T.astype(np.float32)
            wpt[2 * a + b, 64:, :] = wp.T.astype(np.float32)

    bv1 = bv1.astype(np.float32).reshape(128, 1)
    bv2d = np.tile(bv2.astype(np.float32), 2).reshape(128, 1)
    bvhd = np.tile(bvh.astype(np.float32), 2).reshape(128, 1)
    return w1t, bv1, w2t, bv2d, wpt, bvhd


class _Builder:
    def __init__(self, H, W, finalize=True):
        self.finalize = finalize
        import concourse.bass as bass
        import concourse.bacc as bacc
        import concourse.mybir as mybir
        from concourse.dt import dt
        from concourse.tile import TileContext
        from concourse.alu_op_type import AluOpType

        self.bass = bass
        self.bacc = bacc
        self.mybir = mybir
        self.F32, self.F32R = dt.float32, dt.float32r
        self.Act = mybir.ActivationFunctionType
        self.Alu = AluOpType
        self.H, self.W = H, W
        self.BLOCK = 16
        self.NB = H // self.BLOCK
        self.TileContext = TileContext

    def build(self):
        H, W = self.H, self.W
        F32, F32R = self.F32, self.F32R
        HW2 = (H // 2) * (W // 2)
        nc = self.bacc.Bacc(None, target_bir_lowering=False)
        self.nc = nc

        self.params = {}
        for nm, shp, dtp in (
            ("w1t", [9, 64, 128], F32R), ("w2t", [9, 128, 64], F32R),
            ("wpt", [4, 128, 64], F32R), ("bv1", [128, 1], F32),
            ("bv2", [128, 1], F32), ("bvh", [128, 1], F32),
        ):
            self.params[nm] = nc.declare_dram_parameter(nm, shp, dtp,
                                                        isOutput=False)
        # x arrives host-padded: [64, H+2 rows, W+2 cols], zero borders
        # (one col each side, two extra zero rows at the bottom)
        self.x = nc.declare_dram_parameter("x", [64, (H + 2) * (W + 2)], F32R,
                                           isOutput=False)
        xlo = nc.declare_dram_parameter("x_low", [64, HW2], F32, isOutput=True)
        xhi = nc.declare_dram_parameter("x_high", [64, HW2], F32,
                                        isOutput=True)
        self.xlo3 = xlo.rearrange("c (i j) -> c i j", j=W // 2)
        self.xhi3 = xhi.rearrange("c (i j) -> c i j", j=W // 2)

        with self.TileContext(nc) as tc:
            with ExitStack() as st:
                p = {}
                for name, bufs, space in (
                    ("const", 1, "SBUF"), ("xt", 2, "SBUF"),
                    ("h1", 2, "SBUF"), ("z", 3, "SBUF"), ("u", 2, "SBUF"),
                    ("g", 2, "SBUF"), ("r", 2, "SBUF"),
                    ("q", 2, "SBUF"), ("m", 2, "SBUF"), ("h2d", 2, "SBUF"),
                    ("cat", 3, "SBUF"), ("cA", 1, "SBUF"), ("xh", 2, "SBUF"),
                    ("ps1", 4, "PSUM"), ("ps2", 2, "PSUM"),
                    ("psh", 2, "PSUM"),
                ):
                    p[name] = st.enter_context(
                        tc.tile_pool(name=name, bufs=bufs, space=space))
                self.p = p
                self._emit_constants()
                for b in range(self.NB):
                    self._emit_block(b)
        if self.finalize:
            nc.finalize()
        return nc

    def _dram(self, name):
        return self.params[name]

    def _emit_constants(self):
        nc, p = self.nc, self.p
        F32, F32R = self.F32, self.F32R
        self.w1s = p["const"].tile([64, 9 * 128], F32R, tag="w1s")
        nc.sync.dma_start(
            out=self.w1s.rearrange("k (t m) -> k t m", m=128),
            in_=self._dram("w1t").rearrange("t k m -> k t m"))
        self.w2s = p["const"].tile([128, 9 * 64], F32R, tag="w2s")
        nc.sync.dma_start(
            out=self.w2s.rearrange("k (t m) -> k t m", m=64),
            in_=self._dram("w2t").rearrange("t k m -> k t m"))
        self.wps = p["const"].tile([128, 4 * 64], F32R, tag="wps")
        nc.sync.dma_start(
            out=self.wps.rearrange("k (t m) -> k t m", m=64),
            in_=self._dram("wpt").rearrange("t k m -> k t m"))
        self.bv1s = p["const"].tile([128, 1], F32, tag="bv1s")
        nc.sync.dma_start(out=self.bv1s[:], in_=self._dram("bv1")[:])
        self.bv2s = p["const"].tile([128, 1], F32, tag="bv2s")
        nc.sync.dma_start(out=self.bv2s[:], in_=self._dram("bv2")[:])
        self.bvhs = p["const"].tile([128, 1], F32, tag="bvhs")
        nc.sync.dma_start(out=self.bvhs[:], in_=self._dram("bvh")[:])

    def _mish_q(self, z, cols, part=128):
        nc, p, W = self.nc, self.p, self.W
        F32, Act, Alu = self.F32, self.Act, self.Alu
        u = p["u"].tile([128, 4 * W], F32, tag="u")
        nc.scalar.activation(u[0:part, :cols], z[0:part, :cols], Act.Exp)
        g = p["g"].tile([128, 4 * W], F32, tag="g")
        nc.scalar.activation(g[0:part, :cols], u[0:part, :cols], Act.Square,
                             bias=1.0)
        # den reuses the u tile (u is dead once g is computed)
        nc.gpsimd.tensor_scalar_add(u[0:part, :cols], g[0:part, :cols], 1.0)
        r = p["r"].tile([128, 4 * W], F32, tag="r")
        nc.vector.reciprocal_approx_fast(r[0:part, :cols], u[0:part, :cols])
        q = p["q"].tile([128, 4 * W], F32, tag="q")
        nc.vector.scalar_tensor_tensor(
            q[0:part, :cols], g[0:part, :cols], -1.0, r[0:part, :cols],
            Alu.add, Alu.mult)
        return q

    def _emit_mms(self, mms):
        for i, (o, l, rr) in enumerate(mms):
            self.nc.tensor.matmul(o, l, rr, start=(i == 0),
                                  stop=(i == len(mms) - 1))

    def _conv1_group(self, a, n, psum, xtv, rx0):
        # xtv is the zero-bordered [64|128, rows, W+2] view of x rows
        H, W = self.H, self.W
        pv = psum.rearrange("p (rr c) -> p rr c", c=W)
        mms = []
        for dy in (0, -1, 1):
            for dx in (0, 1, -1):
                rows = [rr for rr in range(a, a + n) if 0 <= rr + dy <= H - 1]
                if not rows:
                    continue
                t = (dy + 1) * 3 + (dx + 1)
                i0, nr = rows[0] - a, len(rows)
                psl = pv[:, i0:i0 + nr, :]
                rsl = xtv[0:64, rows[0] + dy - rx0:rows[0] + dy - rx0 + nr,
                          dx + 1:dx + 1 + W]
                mms.append((psl, self.w1s[:, t * 128:(t + 1) * 128], rsl))
        self._emit_mms(mms)

    def _conv2_group(self, ya, psum, h1v, a0):
        H, W = self.H, self.W
        pv = psum.rearrange("p (rr c) -> p rr c", c=W)
        mms = []
        for dy in (0, -1, 1):
            for dx in (0, 1, -1):
                rows = [rr for rr in (ya, ya + 1) if 0 <= rr + dy <= H - 1]
                if not rows:
                    continue
                t = (dy + 1) * 3 + (dx + 1)
                i0, nr = rows[0] - ya, len(rows)
                psl = pv[:, i0:i0 + nr, :]
                rsl = h1v[:, rows[0] + dy - a0:rows[0] + dy - a0 + nr,
                          dx + 1:dx + 1 + W]
                mms.append((psl, self.w2s[:, t * 64:(t + 1) * 64], rsl))
        self._emit_mms(mms)

    def _emit_block(self, b):
        nc, p = self.nc, self.p
        H, W, BLOCK, NB = self.H, self.W, self.BLOCK, self.NB
        F32, F32R, Act, Alu = self.F32, self.F32R, self.Act, self.Alu
        Wh = W // 2
        r0 = b * BLOCK
        a0 = 0 if b == 0 else r0 - 1
        a1 = min(r0 + BLOCK, H - 1)
        groups = []
        a = a0
        while a <= a1:
            n = 2 if a + 1 <= a1 else 1
            groups.append((a, n))
            a += n
        rx0 = max(a0 - 1, 0)
        rx1 = min(a1 + 1, H - 1)
        nxr = rx1 - rx0 + 1

        Wp = W + 2  # zero border column on each side of every row
        xt = p["xt"].tile([64, 20 * Wp], F32R, tag="xt")
        xtv = xt.rearrange("p (rr c) -> p rr c", c=Wp)
        nc.sync.dma_start(
            out=xt[0:64, 0:nxr * Wp],
            in_=self.x[:, rx0 * Wp:(rx1 + 1) * Wp])
        xtvf = xt.bitcast(F32).rearrange("p (rr c) -> p rr c", c=Wp)

        # ---- conv1 -> h1 ----
        h1 = p["h1"].tile([128, 18 * Wp], F32R, tag="h1")
        h1v = h1.rearrange("p (rr c) -> p rr c", c=Wp)
        # zero border columns (both memsets on gpsimd -> one wait for PE)
        h1vf = h1.bitcast(F32).rearrange("p (rr c) -> p rr c", c=Wp)
        nc.gpsimd.memset(h1vf[:, 0:18, 0:1], 0.0)
        nc.gpsimd.memset(h1vf[:, 0:18, W + 1:W + 2], 0.0)
        spans = [groups[i:i + 2] for i in range(0, len(groups), 2)]
        for span in spans:
            cols = sum(n for (_, n) in span) * W
            z = p["z"].tile([128, 4 * W], F32, tag="z")
            off = 0
            for (ga, gn) in span:
                psum = p["ps1"].tile([128, 2 * W], F32, tag="ps1")
                self._conv1_group(ga, gn, psum, xtv, rx0)
                nc.scalar.activation(z[:, off:off + gn * W],
                                     psum[:, 0:gn * W],
                                     Act.Identity, bias=self.bv1s[:])
                off += gn * W
            q = self._mish_q(z, cols)
            lr0 = span[0][0] - a0
            nrows = cols // W
            nc.vector.tensor_mul(out=h1v[:, lr0:lr0 + nrows, 1:W + 1],
                                 in0=z[:, :cols], in1=q[:, :cols])

        # ---- conv2 + bias + mish + residual -> h2d (row-deinterleaved) ----
        # h2d: 16 rows at partitions 0-63, each row stored [evens | odds]
        h2d = p["h2d"].tile([64, BLOCK * W], F32R, tag="h2d")
        for sp in range(4):
            z2 = p["z"].tile([128, 4 * W], F32, tag="z")
            for k in range(2):
                ya = r0 + (sp * 2 + k) * 2
                psum = p["ps2"].tile([64, 2 * W], F32, tag="ps2")
                self._conv2_group(ya, psum, h1v, a0)
                nc.scalar.activation(z2[0:64, k * 2 * W:(k + 1) * 2 * W],
                                     psum[:, 0:2 * W],
                                     Act.Identity, bias=self.bv2s[0:64])
            q2 = self._mish_q(z2, 4 * W, part=64)
            for k in range(2):
                gi = sp * 2 + k
                ya = r0 + gi * 2
                mm = p["m"].tile([64, 2 * W], F32, tag="m")
                nc.vector.tensor_mul(out=mm[:],
                                     in0=z2[0:64, k * 2 * W:(k + 1) * 2 * W],
                                     in1=q2[0:64, k * 2 * W:(k + 1) * 2 * W])
                dout = h2d[:, gi * 2 * W:(gi + 1) * 2 * W].rearrange(
                    "p (rr pp j) -> p rr j pp", rr=2, pp=2, j=Wh)
                nc.vector.tensor_add(
                    out=dout, in0=mm[:],
                    in1=xtvf[0:64, ya - rx0:ya - rx0 + 2, 1:W + 1])

        # ---- DWT low band ----
        # pair i: rows (2i, 2i+1); A/B = row 2i evens/odds, C/D = row 2i+1
        h2f = h2d.bitcast(F32)
        hv = h2f.rearrange("p (pr two t) -> p pr two t", two=2, t=W)
        NP = BLOCK // 2  # pairs per block
        t1 = p["cat"].tile([64, NP * Wh], F32, tag="cat")
        t1v = t1.rearrange("p (pr j) -> p pr j", j=Wh)
        nc.vector.tensor_add(out=t1v, in0=hv[:, :, 0, 0:Wh],
                             in1=hv[:, :, 0, Wh:W])
        t2 = p["cat"].tile([64, NP * Wh], F32, tag="cat")
        t2v = t2.rearrange("p (pr j) -> p pr j", j=Wh)
        nc.vector.tensor_add(out=t2v, in0=hv[:, :, 1, 0:Wh],
                             in1=hv[:, :, 1, Wh:W])
        s = p["cat"].tile([64, NP * Wh], F32, tag="cat")
        nc.vector.tensor_add(out=s[:], in0=t1[:], in1=t2[:])
        cat = p["cA"].tile([64, NP * Wh], F32, tag="cA")
        nc.gpsimd.tensor_scalar_mul(cat[:], s[:], 0.5)
        nc.sync.dma_start(
            out=self.xlo3[:, NP * b:NP * (b + 1), :],
            in_=cat.rearrange("c (pr j) -> c pr j", j=Wh))

        # ---- convh: fused DWT-high + 1x1 conv + mish ----
        # psum group hg covers pairs (2hg, 2hg+1); tap (a,b) reads
        # h2d rows (2i+a) at col offset b*Wh
        zh = p["z"].tile([128, 4 * W], F32, tag="z")
        h2r = h2d.rearrange("p (pr two bb j) -> p pr two bb j",
                            two=2, bb=2, j=Wh)
        for hg in range(NP // 2):
            psum = p["psh"].tile([64, W], F32, tag="psh")
            mms = []
            for t4 in range(4):
                aa, bb = t4 // 2, t4 % 2
                rsl = h2r[:, 2 * hg:2 * hg + 2, aa, bb, :]
                psl = psum[:, :]
                mms.append((psl, self.wps[0:64, t4 * 64:(t4 + 1) * 64], rsl))
            self._emit_mms(mms)
            nc.scalar.activation(zh[0:64, hg * W:(hg + 1) * W], psum[:],
                                 Act.Identity, bias=self.bvhs[0:64])
        qh = self._mish_q(zh, (NP // 2) * W, part=64)
        xht = p["xh"].tile([64, NP * Wh], F32, tag="xh")
        nc.vector.tensor_mul(out=xht[:], in0=zh[0:64, :(NP // 2) * W],
                             in1=qh[0:64, :(NP // 2) * W])
        nc.sync.dma_start(
            out=self.xhi3[:, NP * b:NP * (b + 1), :],
            in_=xht.rearrange("c (pr j) -> c pr j", j=Wh))


def _build(H, W, finalize=True):
    return _Builder(H, W, finalize=finalize).build()


def _get_program(H, W):
    key = (H, W)
    if key not in _CACHE:
        _CACHE[key] = _build(H, W)
    return _CACHE[key]


def kernel(x, w1, b1, g1, be1, m1, v1, w2, b2, g2, be2, m2, v2,
           wh, bh, gh, beh, mh, vh):
    from concourse.bass_utils import run_bass_kernel_spmd

    x = np.asarray(x, dtype=np.float32)
    B, C, H, W = x.shape
    w1t, bv1, w2t, bv2d, wpt, bvhd = _fold_params(
        np.asarray(w1, np.float32), np.asarray(b1, np.float32),
        np.asarray(g1, np.float32), np.asarray(be1, np.float32),
        np.asarray(m1, np.float32), np.asarray(v1, np.float32),
        np.asarray(w2, np.float32), np.asarray(b2, np.float32),
        np.asarray(g2, np.float32), np.asarray(be2, np.float32),
        np.asarray(m2, np.float32), np.asarray(v2, np.float32),
        np.asarray(wh, np.float32), np.asarray(bh, np.float32),
        np.asarray(gh, np.float32), np.asarray(beh, np.float32),
        np.asarray(mh, np.float32), np.asarray(vh, np.float32))

    nc = _get_program(H, W)
    core_ids = list(range(B))
    xp = np.zeros((B, C, H + 2, W + 2), dtype=np.float32)
    xp[:, :, 0:H, 1:W + 1] = x
    in_maps = []
    for i in range(B):
        in_maps.append({
            "x": np.ascontiguousarray(xp[i].reshape(C, (H + 2) * (W + 2))),
            "w1t": w1t, "w2t": w2t, "wpt": wpt,
            "bv1": bv1, "bv2": bv2d, "bvh": bvhd,
        })
    trace = os.environ.get("KERNEL_TRACE", "0") == "1"
    try:
        res = run_bass_kernel_spmd(nc, in_maps, core_ids, trace=trace)
    except ModuleNotFoundError:
        # NTFF trace hook unavailable in this container
        res = run_bass_kernel_spmd(nc, in_maps, core_ids, trace=False)
    if res.exec_time_ns is not None:
        print(f"HW exec time: {res.exec_time_ns} ns")
    H2, W2 = H // 2, W // 2
    x_low = np.stack([res.results[i]["x_low"].reshape(C, H2, W2)
                      for i in range(B)])
    x_high = np.stack([res.results[i]["x_high"].reshape(C, H2, W2)
                       for i in range(B)])
    return (x_low, x_high)
